# revision 12
# baseline (speedup 1.0000x reference)
"""Trainium2 Bass kernel for relational GNN message passing (SpMM).

Computes: out = weight[idx] * segment_sum(edge_vals[idx][:,None] * x[edge_cols[idx]],
                                          edge_rows[idx], N)

Strategy (8 NeuronCores, SPMD — one program, per-core data):
- Host: destination rows are processed in 128-row ranges.  The ranges are
  sorted by edge count and dealt round-robin to (core, block) slots, so the 8
  cores' buckets for a given block index have near-equal counts and the
  shared chunk schedule M[b, h] = ceil(max_core_count / 128) carries minimal
  padding.  Edges are bucketed by (core, block, source half) — dma_gather
  indices are int16, so sources are addressed as x_lo = x[:N/2], x_hi.
- x is converted to bf16 and padded to 128 elements per row (256B — the
  dma_gather elem granularity), so gathered tiles are matmul-ready bf16.
- One dma_gather call per (block, half) bucket, round-robin over the 4 SWDGE
  queues; pad slots gather row 0 and carry zero sel entries.
- The per-chunk selection matrices sel[e, d] = vals[e] * (rows_rel[e] == d)
  are precomputed on the host in bf16 (pure edge-metadata marshalling) and
  streamed in with large sequential DMAs.  No DVE work on the critical path.
- Device (per core): the tensor engine accumulates psum[64 feat, 128 dst] +=
  xs[128e, :64].T @ sel[128e, 128] over a block's chunks (bf16 matmul, fp32
  psum).  Eviction multiplies by weight[idx] (ACT engine, per-partition
  scale) into an SBUF stage; the stage is written back in groups of blocks.
- Host: scatter each core's out_t columns back to global rows by range.
"""

import sys

for _p in ("/opt/trn_rl_repo",):
    if _p not in sys.path:
        sys.path.insert(0, _p)

from contextlib import ExitStack

import numpy as np

from concourse import bacc, mybir, tile
from concourse.bass_utils import run_bass_kernel_spmd

P = 128           # partitions / edges per chunk / dst rows per block
NCORES = 8
XPAD = 128        # bf16 elements per padded x row (256B)
OUTGRP = 7        # blocks per output writeback DMA
SPLITB = 3        # blocks whose idx loads in the small head DMA

BF16 = mybir.dt.bfloat16

# Set by test.py to capture an NTFF profile; harness leaves these alone.
TRACE = False
TRACE_DIR = None
LAST_EXEC_NS = None

_PROGRAM_CACHE = {}


def _build_program(D, n_lo_src, n_hi_src, M, out_cols):
    """Build the SPMD Bass program for chunk schedule M[NBLK, 2]."""
    NBLK = M.shape[0]
    C_tot = int(M.sum())
    MAXCH = int(M.max())
    flat = M.reshape(-1)
    cb = np.concatenate(([0], np.cumsum(flat)[:-1])).reshape(NBLK, 2)
    CB0 = int(cb[SPLITB, 0])  # chunk base of the idx tail DMA

    nc = bacc.Bacc("TRN2", target_bir_lowering=False, debug=False,
                   num_devices=NCORES, num_swdge_queues=4)

    x_lo_d = nc.dram_tensor("x_lo", [n_lo_src, XPAD], BF16,
                            kind="ExternalInput")
    x_hi_d = nc.dram_tensor("x_hi", [n_hi_src, XPAD], BF16,
                            kind="ExternalInput")
    idx_d = nc.dram_tensor("idx", [P, C_tot * 8], mybir.dt.int16,
                           kind="ExternalInput")
    sel_d = nc.dram_tensor("sel", [P, C_tot * P], BF16, kind="ExternalInput")
    w_d = nc.dram_tensor("w", [P, 1], mybir.dt.float32, kind="ExternalInput")
    out_d = nc.dram_tensor("out_t", [D, out_cols], mybir.dt.float32,
                           kind="ExternalOutput")

    x_src = {0: x_lo_d, 1: x_hi_d}

    with tile.TileContext(nc) as tc, ExitStack() as ctx:
        const = ctx.enter_context(tc.tile_pool(name="const", bufs=1))
        xs_pool = ctx.enter_context(tc.tile_pool(name="xs", bufs=16))
        sel_pool = ctx.enter_context(tc.tile_pool(name="selp", bufs=12))
        psum = ctx.enter_context(tc.tile_pool(name="psum", bufs=4,
                                              space="PSUM"))
        outp = ctx.enter_context(tc.tile_pool(name="outp", bufs=1))

        w_t = const.tile([P, 1], mybir.dt.float32, tag="w")
        nc.sync.dma_start(out=w_t[:], in_=w_d[:])

        # head idx slice first so the first gathers start early
        idx_a = const.tile([P, CB0 * 8], mybir.dt.int16, tag="idxa")
        nc.sync.dma_start(out=idx_a[:], in_=idx_d[:, : CB0 * 8])
        idx_b = const.tile([P, (C_tot - CB0) * 8], mybir.dt.int16, tag="idxb")
        nc.sync.dma_start(out=idx_b[:], in_=idx_d[:, CB0 * 8 :])

        stage = outp.tile([P, out_cols], mybir.dt.float32, tag="stage")

        gcount = [0]

        def issue_gather(b, h):
            nch = int(M[b, h])
            base = int(cb[b, h])
            if base < CB0:
                it = idx_a[:, base * 8 : (base + nch) * 8]
            else:
                it = idx_b[:, (base - CB0) * 8 : (base - CB0 + nch) * 8]
            t = xs_pool.tile([P, MAXCH, XPAD], BF16, tag="xs",
                             name=f"xs_{b}_{h}")
            nc.gpsimd.dma_gather(
                t[:, :nch, :],
                x_src[h][:],
                it,
                nch * P,
                nch * P,
                XPAD,
                single_packet=False,
                queue_num=gcount[0] % 4,
            )
            gcount[0] += 1
            st = sel_pool.tile([P, MAXCH, P], BF16, tag="sel",
                               name=f"sel_{b}_{h}")
            nc.sync.dma_start(
                out=st[:, :nch, :],
                in_=sel_d[:, base * P : (base + nch) * P].rearrange(
                    "p (c d) -> p c d", d=P),
            )
            return t, st

        tiles = {}
        for b in range(NBLK):
            for h in (0, 1):
                if int(M[b, h]):
                    tiles[(b, h)] = issue_gather(b, h)

            nch = int(M[b, 0] + M[b, 1])
            ps = psum.tile([D, P], mybir.dt.float32, space="PSUM", tag="ps")
            k = 0
            for h in (0, 1):
                for c in range(int(M[b, h])):
                    xs_t, sel_t = tiles[(b, h)]
                    nc.tensor.matmul(
                        ps[:],
                        lhsT=xs_t[:, c, 0:D],
                        rhs=sel_t[:, c, :],
                        start=(k == 0),
                        stop=(k == nch - 1),
                    )
                    k += 1
            nc.scalar.activation(
                out=stage[:D, b * P : (b + 1) * P],
                in_=ps[:],
                func=mybir.ActivationFunctionType.Copy,
                scale=w_t[:D, 0:1],
            )
            if b % OUTGRP == OUTGRP - 1 or b == NBLK - 1:
                lo = (b - b % OUTGRP) * P
                hi = (b + 1) * P
                nc.sync.dma_start(out=out_d[:, lo:hi],
                                  in_=stage[:D, lo:hi])

    nc.compile()
    return nc


def kernel(x, weight, edge_vals, edge_rows, edge_cols, idx):
    global LAST_EXEC_NS

    x = np.ascontiguousarray(np.asarray(x, dtype=np.float32))
    weight = np.asarray(weight, dtype=np.float32)
    i = int(np.asarray(idx))
    rows = np.asarray(edge_rows[i], dtype=np.int64)
    cols = np.asarray(edge_cols[i], dtype=np.int64)
    vals = np.asarray(edge_vals[i], dtype=np.float32)

    N, D = x.shape
    E = rows.shape[0]
    assert D == 64, D
    NRNG = -(-N // P)              # 128-row dst ranges
    NBLK = -(-NRNG // NCORES)      # blocks per core
    NSLOT = NCORES * NBLK
    XH = -(-N // 2)                # source half size
    assert XH <= 32768, XH
    n_hi_src = N - XH

    bf16 = np.dtype(mybir.dt.np(BF16))
    x_pad = np.zeros((N, XPAD), dtype=bf16)
    x_pad[:, :D] = x.astype(bf16)

    # ---- host prep -------------------------------------------------------
    rblk = rows >> 7                                   # global range id
    half = (cols >= XH).astype(np.int64)

    # balance: sort ranges by edge count, deal groups of 8 to the cores
    tot = np.bincount(rblk, minlength=NSLOT)
    order_r = np.argsort(-tot, kind="stable")
    rank_of_range = np.empty(NSLOT, np.int64)
    rank_of_range[order_r] = np.arange(NSLOT)
    blk_of_range = rank_of_range // NCORES
    core_of_range = rank_of_range % NCORES

    core = core_of_range[rblk]
    block = blk_of_range[rblk]
    ngrp = NCORES * NBLK * 2
    key = (core * NBLK + block) * 2 + half
    order = np.argsort(key, kind="stable")
    ks = key[order]
    cnt = np.bincount(ks, minlength=ngrp)
    starts = np.concatenate(([0], np.cumsum(cnt)[:-1]))
    within = np.arange(E, dtype=np.int64) - starts[ks]

    # chunk schedule: max over cores, shared by the SPMD program
    cnt_cbh = cnt.reshape(NCORES, NBLK, 2)
    M = -(-cnt_cbh.max(axis=0) // P)           # [NBLK, 2]
    M[(M.sum(axis=1) == 0), 0] = 1             # empty block -> one pad chunk
    C_tot = int(M.sum())
    flat = M.reshape(-1)
    call_base = np.concatenate(([0], np.cumsum(flat)[:-1])) * P  # [NBLK*2]
    slot_off_g = np.tile(call_base, NCORES)
    slots = slot_off_g[ks] + within

    rows_rel_s = (rows[order] & 127).astype(np.int64)
    colh_s = (cols[order] - XH * half[order]).astype(np.int16)
    vals_s = vals[order].astype(bf16)
    core_s = ks // (NBLK * 2)

    wvec = np.full((P, 1), weight[i], np.float32)
    out_cols = NBLK * P

    def pack_idx(flat16, C):
        # logical slot j -> [j % 16, j // 16], replicated across 8 Q7 cores
        a = flat16.reshape(C * 8, 16).T
        return np.tile(a, (NCORES, 1))

    in_maps = []
    for c in range(NCORES):
        m = core_s == c
        sh = slots[m]
        # pads: idx = 0 (gathered but multiplied away by the zero sel rows)
        i16 = np.zeros(C_tot * P, np.int16)
        i16[sh] = colh_s[m]
        sel_flat = np.zeros((C_tot * P, P), dtype=bf16)
        sel_flat[sh, rows_rel_s[m]] = vals_s[m]
        sel_pc = sel_flat.reshape(C_tot, P, P).transpose(1, 0, 2)
        im = {
            "x_lo": x_pad[:XH],
            "x_hi": x_pad[XH:],
            "w": wvec,
            "idx": pack_idx(i16, C_tot),
            "sel": np.ascontiguousarray(sel_pc).reshape(P, C_tot * P),
        }
        in_maps.append(im)

    # ---- build / fetch program ------------------------------------------
    sig = (D, XH, n_hi_src, out_cols, M.tobytes())
    if sig not in _PROGRAM_CACHE:
        _PROGRAM_CACHE[sig] = _build_program(D, XH, n_hi_src, M, out_cols)
    nc = _PROGRAM_CACHE[sig]

    kw = {}
    if TRACE:
        kw = dict(trace=True, tmpdir=TRACE_DIR)
    res = run_bass_kernel_spmd(nc, in_maps, list(range(NCORES)), **kw)
    LAST_EXEC_NS = res.exec_time_ns

    out = np.empty((N, D), np.float32)
    outs = [res.results[c]["out_t"].T for c in range(NCORES)]  # [cols, D]
    for rid in range(NRNG):
        c = int(core_of_range[rid])
        b = int(blk_of_range[rid])
        lo = rid * P
        hi = min(lo + P, N)
        out[lo:hi] = outs[c][b * P : b * P + (hi - lo)]
    return out
